# revision 12
# baseline (speedup 1.0000x reference)
"""Trainium2 Bass kernel for EnhancedGraphGenerator (GAT-style pairwise scorer).

Math (reference):
    h   = relu(x @ W1 + b1) @ W2 + b2                       # [N, H]
    e_i = h @ Wa1[:H]; e_j = h @ Wa1[H:]                    # [N, H]
    z   = relu(e_i[:,None,:] + e_j[None,:,:] + ba1)         # [N, N, H]
    s   = z . wa2 + ba2; s /= clip(t, .1, 5); s = (s+s.T)/2
    adj = sigmoid(s);  loss = 0.01 * mean(|adj|)

Distribution: row-sharded across 8 cores; h/e replicated (cheap), each core
computes both (i,j) and (j,i) raw scores for its 128 rows locally.

Device layout: h on partitions (64, duplicated to 128 for an i-pair), node
index on free dim. Pre-activation relu(e_jT + bias_col) is one fused
tensor_scalar/activation op over [128, 1024] in bf16 (DVE 4x mode); spread
over DVE/GPSIMD/ACT. The wa2 dot is a bf16 PE matmul with stationary
[128, 32] weights (wa2 in cols 0/1, zeros elsewhere -> psum rows 2..31
zeroed for free). tile_position col-tiling packs 4 i-pair blocks per PSUM
bank pair; pass A starts, pass B accumulates (same col group -> same bank
partitions, no cross-group has_written hazard). One ACT sigmoid per quad
evacuates into a persistent sig_all staging tile; 8 big strided DMAs write
the final adjacency rows.
"""

import sys

sys.path.insert(0, "/opt/trn_rl_repo")

import numpy as np

N, F, H = 1024, 512, 64
NCORES = 8
BLK = N // NCORES  # 128 rows per core
SPARSITY_WEIGHT = 0.01

_cache = {}

# producer engine schedule (weighted round robin): v=DVE, g=GPSIMD, a=ACT
N_PROD, N_ACT = 128, 38
PROD_PATTERN = "".join(
    "a" if (i * N_ACT) // N_PROD != ((i + 1) * N_ACT) // N_PROD else "v"
    for i in range(N_PROD))


def _build_program():
    import concourse.bacc as bacc
    import concourse.tile as tile
    from concourse import mybir

    f32 = mybir.dt.float32
    bf16 = mybir.dt.bfloat16
    AF = mybir.ActivationFunctionType
    OP = mybir.AluOpType

    nc = bacc.Bacc("TRN2", target_bir_lowering=False, debug=False)

    # ---- DRAM I/O ----
    xT_d = nc.dram_tensor("xT", [F, N], f32, kind="ExternalInput")
    xbT_d = nc.dram_tensor("xbT", [F, BLK], f32, kind="ExternalInput")
    w1_d = nc.dram_tensor("w1", [F, H], f32, kind="ExternalInput")
    w2_d = nc.dram_tensor("w2", [H, H], f32, kind="ExternalInput")
    wa1lo_d = nc.dram_tensor("wa1lo", [H, 128], f32, kind="ExternalInput")
    wa1hi_d = nc.dram_tensor("wa1hi", [H, 128], f32, kind="ExternalInput")
    wa2c_d = nc.dram_tensor("wa2c", [128, 32], bf16, kind="ExternalInput")
    b1_d = nc.dram_tensor("b1c", [H, 1], f32, kind="ExternalInput")
    b2_d = nc.dram_tensor("b2c", [H, 1], f32, kind="ExternalInput")
    ba12_d = nc.dram_tensor("ba12", [128, 1], f32, kind="ExternalInput")
    scol_d = nc.dram_tensor("scol", [128, 1], f32, kind="ExternalInput")
    sbcol_d = nc.dram_tensor("sbcol", [128, 1], f32, kind="ExternalInput")
    adj_d = nc.dram_tensor("adj_blk", [BLK, N], f32, kind="ExternalOutput")

    with tile.TileContext(nc) as tc:
        with (
            tc.tile_pool(name="consts", bufs=1) as cp,
            tc.tile_pool(name="pre", bufs=12) as pp,
            tc.tile_pool(name="psum0", bufs=2, space="PSUM") as ps0,
            tc.tile_pool(name="psumM", bufs=3, space="PSUM") as psM,
        ):
            # ---- const / input loads ----
            w1t = cp.tile([128, 4 * H], f32)
            for c in range(4):
                nc.sync.dma_start(w1t[:, c * H:(c + 1) * H],
                                  w1_d[c * 128:(c + 1) * 128, :])
            w2t = cp.tile([H, H], f32)
            nc.sync.dma_start(w2t[:], w2_d[:])
            wa1lo = cp.tile([H, 128], f32)
            nc.sync.dma_start(wa1lo[:], wa1lo_d[:])
            wa1hi = cp.tile([H, 128], f32)
            nc.sync.dma_start(wa1hi[:], wa1hi_d[:])
            wa2c = cp.tile([128, 32], bf16)
            nc.sync.dma_start(wa2c[:], wa2c_d[:])
            b1c = cp.tile([H, 1], f32)
            nc.sync.dma_start(b1c[:], b1_d[:])
            b2c = cp.tile([H, 1], f32)
            nc.sync.dma_start(b2c[:], b2_d[:])
            ba12 = cp.tile([128, 1], f32)
            nc.sync.dma_start(ba12[:], ba12_d[:])
            scol = cp.tile([128, 1], f32)
            nc.sync.dma_start(scol[:], scol_d[:])
            sbcol = cp.tile([128, 1], f32)
            nc.sync.dma_start(sbcol[:], sbcol_d[:])

            xt = cp.tile([128, 4 * N], f32)
            for c in range(4):
                for hh in range(2):
                    eng = nc.sync if (2 * c + hh) % 2 == 0 else nc.gpsimd
                    eng.dma_start(
                        xt[:, c * N + hh * 512:c * N + (hh + 1) * 512],
                        xT_d[c * 128:(c + 1) * 128, hh * 512:(hh + 1) * 512])
            xbt = cp.tile([128, 4 * BLK], f32)
            for c in range(4):
                eng = nc.sync if c % 2 == 0 else nc.gpsimd
                eng.dma_start(xbt[:, c * BLK:(c + 1) * BLK],
                              xbT_d[c * 128:(c + 1) * 128, :])

            # ---- stage 0, j-side: e_jT / e_iT over all N, duplicated 2x on
            # partitions (rows 0-63 and 64-127 both hold the 64 h-dims) ----
            h1T = cp.tile([H, N], f32)
            hT = cp.tile([H, N], f32)
            ejT2 = cp.tile([128, N], f32)
            eiT2 = cp.tile([128, N], f32)
            for jh in range(2):
                s = slice(512 * jh, 512 * (jh + 1))
                ps1 = ps0.tile([H, 512], f32, tag="s0")
                for c in range(4):
                    nc.tensor.matmul(ps1[:], w1t[:, c * H:(c + 1) * H],
                                     xt[:, c * N + 512 * jh: c * N + 512 * (jh + 1)],
                                     start=(c == 0), stop=(c == 3))
                nc.scalar.activation(h1T[:, s], ps1[:], AF.Relu, bias=b1c[:])
                ps2 = ps0.tile([H, 512], f32, tag="s0")
                nc.tensor.matmul(ps2[:], w2t[:], h1T[:, s], start=True, stop=True)
                nc.scalar.activation(hT[:, s], ps2[:], AF.Identity, bias=b2c[:])
                psj = ps0.tile([128, 512], f32, tag="s0")
                nc.tensor.matmul(psj[:], wa1hi[:], hT[:, s], start=True, stop=True)
                nc.vector.tensor_copy(ejT2[:, s], psj[:])
                psi = ps0.tile([128, 512], f32, tag="s0")
                nc.tensor.matmul(psi[:], wa1lo[:], hT[:, s], start=True, stop=True)
                nc.vector.tensor_copy(eiT2[:, s], psi[:])

            # ---- stage 0, i-side (this core's 128 rows, from xbT) ----
            h1Tb = cp.tile([H, BLK], f32)
            hTb = cp.tile([H, BLK], f32)
            eiTb2p = cp.tile([128, BLK], f32)
            ejTb2p = cp.tile([128, BLK], f32)
            psb1 = ps0.tile([H, BLK], f32, tag="s0")
            for c in range(4):
                nc.tensor.matmul(psb1[:], w1t[:, c * H:(c + 1) * H],
                                 xbt[:, c * BLK:(c + 1) * BLK],
                                 start=(c == 0), stop=(c == 3))
            nc.scalar.activation(h1Tb[:], psb1[:], AF.Relu, bias=b1c[:])
            psb2 = ps0.tile([H, BLK], f32, tag="s0")
            nc.tensor.matmul(psb2[:], w2t[:], h1Tb[:], start=True, stop=True)
            nc.scalar.activation(hTb[:], psb2[:], AF.Identity, bias=b2c[:])
            psbe = ps0.tile([128, BLK], f32, tag="s0")
            nc.tensor.matmul(psbe[:], wa1lo[:], hTb[:], start=True, stop=True)
            nc.scalar.activation(eiTb2p[:], psbe[:], AF.Identity, bias=ba12[:])
            psbe2 = ps0.tile([128, BLK], f32, tag="s0")
            nc.tensor.matmul(psbe2[:], wa1hi[:], hTb[:], start=True, stop=True)
            nc.scalar.activation(ejTb2p[:], psbe2[:], AF.Identity, bias=ba12[:])

            # bias columns: biasA[:, b] = [e_i(2b)+ba1 ; e_i(2b+1)+ba1]
            biasA = cp.tile([128, 64], f32)
            biasB = cp.tile([128, 64], f32)
            nc.vector.tensor_copy(biasA[0:64, :], eiTb2p[0:64, 0:128:2])
            nc.vector.tensor_copy(biasA[64:128, :], eiTb2p[64:128, 1:128:2])
            nc.vector.tensor_copy(biasB[0:64, :], ejTb2p[0:64, 0:128:2])
            nc.vector.tensor_copy(biasB[64:128, :], ejTb2p[64:128, 1:128:2])

            # persistent staging for all 16 quads' sigmoid outputs
            sig_all = cp.tile([128, 16 * N], f32)

            # ---- main loop: 16 quads x 4 col-groups x (A,B) passes ----
            prod_idx = 0

            def producer(out_t, in_t, bias_col):
                nonlocal prod_idx
                kind = PROD_PATTERN[prod_idx % len(PROD_PATTERN)]
                if kind == "a":
                    nc.scalar.activation(out_t[:], in_t[:], AF.Relu,
                                         bias=bias_col)
                else:
                    nc.vector.tensor_scalar(out_t[:], in_t[:], bias_col, 0.0,
                                            OP.add, OP.max)
                prod_idx += 1

            for q in range(16):
                pq = psM.tile([128, N], f32, tag="pq")
                pres = []
                for g in range(4):
                    b = 4 * q + g
                    preA = pp.tile([128, N], bf16, tag="pre")
                    producer(preA, ejT2, biasA[:, b:b + 1])
                    preB = pp.tile([128, N], bf16, tag="pre")
                    producer(preB, eiT2, biasB[:, b:b + 1])
                    pres.append((preA, preB))
                # interleave col groups for PE concurrency; A starts the
                # accumulation group of its (col-group, bank), B closes it
                # sim's group tracker is partition-base blind -> false
                # collisions across col groups; HW has_written is
                # per-partition and each 512-f32 MM consumes its whole
                # 2KB zero region, so skipping the check is sound here
                for jh in range(2):
                    s = slice(512 * jh, 512 * (jh + 1))
                    for g in range(4):
                        nc.tensor.matmul(pq[32 * g:32 * g + 32, s], wa2c[:],
                                         pres[g][0][:, s], start=True,
                                         stop=False, tile_position=(0, 32 * g),
                                         skip_group_check=True)
                for jh in range(2):
                    s = slice(512 * jh, 512 * (jh + 1))
                    for g in range(4):
                        nc.tensor.matmul(pq[32 * g:32 * g + 32, s], wa2c[:],
                                         pres[g][1][:, s], start=False,
                                         stop=True, tile_position=(0, 32 * g),
                                         skip_group_check=True)
                nc.scalar.activation(sig_all[:, q * N:(q + 1) * N], pq[:],
                                     AF.Sigmoid, bias=sbcol[:], scale=scol[:])
                # flush half the output mid-run, rest at the end; spread the
                # single-partition reads across idle engine DMA queues
                if q in (7, 15):
                    half = 0 if q == 7 else 1
                    h0 = half * 8 * N
                    dq = [nc.sync, nc.gpsimd]
                    for k, (g, m) in enumerate(
                            (g, m) for g in range(4) for m in range(2)):
                        src = sig_all[32 * g + m:32 * g + m + 1,
                                      h0:h0 + 8 * N]
                        dst = adj_d[half * 64 + 2 * g + m:
                                    half * 64 + 64:8, :]
                        dq[k % 2].dma_start(dst, src)

    nc.compile()
    return nc


def _host_prep(node_features, W1, b1, W2, b2, Wa1, ba1, wa2, ba2, temperature):
    """Host-side input layout prep (transposes / tiling / scalar folding)."""
    import ml_dtypes

    x = np.asarray(node_features, np.float32)
    W1 = np.asarray(W1, np.float32)
    W2 = np.asarray(W2, np.float32)
    Wa1 = np.asarray(Wa1, np.float32)
    b1 = np.asarray(b1, np.float32)
    b2 = np.asarray(b2, np.float32)
    ba1 = np.asarray(ba1, np.float32)
    wa2 = np.asarray(wa2, np.float32)
    ba2 = np.float32(ba2)
    t = float(np.clip(np.float32(temperature), 0.1, 5.0))

    xT = np.ascontiguousarray(x.T)                           # [F, N]
    wa1lo = np.ascontiguousarray(np.tile(Wa1[:H], (1, 2)))   # [64, 128]
    wa1hi = np.ascontiguousarray(np.tile(Wa1[H:], (1, 2)))   # [64, 128]
    wa2c = np.zeros((128, 32), np.float32)
    wa2c[0:64, 0] = wa2
    wa2c[64:128, 1] = wa2
    ba12 = np.tile(ba1, 2).reshape(128, 1)
    scol = np.full((128, 1), 0.5 / t, np.float32)
    sbcol = np.full((128, 1), ba2 / t, np.float32)

    common = {
        "xT": xT,
        "w1": W1,
        "w2": W2,
        "wa1lo": wa1lo,
        "wa1hi": wa1hi,
        "wa2c": wa2c.astype(ml_dtypes.bfloat16),
        "b1c": b1.reshape(H, 1),
        "b2c": b2.reshape(H, 1),
        "ba12": ba12.astype(np.float32),
        "scol": scol,
        "sbcol": sbcol,
    }
    in_maps = []
    for c in range(NCORES):
        m = dict(common)
        m["xbT"] = np.ascontiguousarray(xT[:, c * BLK:(c + 1) * BLK])
        in_maps.append(m)
    return in_maps


def kernel(node_features, W1, b1, W2, b2, Wa1, ba1, wa2, ba2, temperature):
    from concourse.bass_utils import run_bass_kernel_spmd

    if "nc" not in _cache:
        _cache["nc"] = _build_program()
    nc = _cache["nc"]

    in_maps = _host_prep(node_features, W1, b1, W2, b2, Wa1, ba1, wa2, ba2,
                         temperature)
    res = run_bass_kernel_spmd(nc, in_maps, list(range(NCORES)))
    adj = np.concatenate([res.results[c]["adj_blk"] for c in range(NCORES)],
                         axis=0)
    loss = np.float32(SPARSITY_WEIGHT) * np.mean(np.abs(adj), dtype=np.float32)
    return adj, np.float32(loss)


# revision 15
# speedup vs baseline: 1.1366x; 1.1366x over previous
"""Trainium2 Bass kernel for EnhancedGraphGenerator (GAT-style pairwise scorer).

Math (reference):
    h   = relu(x @ W1 + b1) @ W2 + b2                       # [N, H]
    e_i = h @ Wa1[:H]; e_j = h @ Wa1[H:]                    # [N, H]
    z   = relu(e_i[:,None,:] + e_j[None,:,:] + ba1)         # [N, N, H]
    s   = z . wa2 + ba2; s /= clip(t, .1, 5); s = (s+s.T)/2
    adj = sigmoid(s);  loss = 0.01 * mean(|adj|)

Distribution: row-sharded across 8 cores; h/e replicated (cheap), each core
computes both (i,j) and (j,i) raw scores for its 128 rows locally.

Device layout: h on partitions (64, duplicated to 128 for an i-pair), node
index on free dim. Pre-activation relu(e_jT + bias_col) is one fused
tensor_scalar/activation op over [128, 1024] in bf16 (DVE 4x mode); spread
over DVE/GPSIMD/ACT. The wa2 dot is a bf16 PE matmul with stationary
[128, 32] weights (wa2 in cols 0/1, zeros elsewhere -> psum rows 2..31
zeroed for free). tile_position col-tiling packs 4 i-pair blocks per PSUM
bank pair; pass A starts, pass B accumulates (same col group -> same bank
partitions, no cross-group has_written hazard). One ACT sigmoid per quad
evacuates into a persistent sig_all staging tile; 8 big strided DMAs write
the final adjacency rows.
"""

import sys

sys.path.insert(0, "/opt/trn_rl_repo")

import numpy as np

N, F, H = 1024, 512, 64
NCORES = 8
BLK = N // NCORES  # 128 rows per core
SPARSITY_WEIGHT = 0.01

_cache = {}

# producer engine schedule (weighted round robin): v=DVE, g=GPSIMD, a=ACT
N_PROD, N_ACT = 128, 36
PROD_PATTERN = "".join(
    "a" if (i * N_ACT) // N_PROD != ((i + 1) * N_ACT) // N_PROD else "v"
    for i in range(N_PROD))


def _build_program():
    import concourse.bacc as bacc
    import concourse.tile as tile
    from concourse import mybir

    f32 = mybir.dt.float32
    bf16 = mybir.dt.bfloat16
    AF = mybir.ActivationFunctionType
    OP = mybir.AluOpType

    nc = bacc.Bacc("TRN2", target_bir_lowering=False, debug=False)

    # ---- DRAM I/O ----
    xT_d = nc.dram_tensor("xT", [F, N], bf16, kind="ExternalInput")
    xbT_d = nc.dram_tensor("xbT", [F, BLK], bf16, kind="ExternalInput")
    w1_d = nc.dram_tensor("w1", [F, H], bf16, kind="ExternalInput")
    w2_d = nc.dram_tensor("w2", [H, H], bf16, kind="ExternalInput")
    wa1lo_d = nc.dram_tensor("wa1lo", [H, 128], bf16, kind="ExternalInput")
    wa1hi_d = nc.dram_tensor("wa1hi", [H, 128], bf16, kind="ExternalInput")
    wa2c_d = nc.dram_tensor("wa2c", [128, 32], bf16, kind="ExternalInput")
    b1_d = nc.dram_tensor("b1c", [H, 1], f32, kind="ExternalInput")
    b2_d = nc.dram_tensor("b2c", [H, 1], f32, kind="ExternalInput")
    ba12_d = nc.dram_tensor("ba12", [128, 1], f32, kind="ExternalInput")
    scol_d = nc.dram_tensor("scol", [128, 1], f32, kind="ExternalInput")
    sbcol_d = nc.dram_tensor("sbcol", [128, 1], f32, kind="ExternalInput")
    adj_d = nc.dram_tensor("adj_blk", [BLK, N], f32, kind="ExternalOutput")

    with tile.TileContext(nc) as tc:
        with (
            tc.tile_pool(name="consts", bufs=1) as cp,
            tc.tile_pool(name="pre", bufs=12) as pp,
        ):
            ps0_cm = tc.tile_pool(name="psum0", bufs=2, space="PSUM")
            ps0 = ps0_cm.__enter__()
            # ---- const / input loads ----
            w1t = cp.tile([128, 4 * H], bf16)
            for c in range(4):
                nc.sync.dma_start(w1t[:, c * H:(c + 1) * H],
                                  w1_d[c * 128:(c + 1) * 128, :])
            w2t = cp.tile([H, H], bf16)
            nc.sync.dma_start(w2t[:], w2_d[:])
            wa1lo = cp.tile([H, 128], bf16)
            nc.sync.dma_start(wa1lo[:], wa1lo_d[:])
            wa1hi = cp.tile([H, 128], bf16)
            nc.sync.dma_start(wa1hi[:], wa1hi_d[:])
            wa2c = cp.tile([128, 32], bf16)
            nc.sync.dma_start(wa2c[:], wa2c_d[:])
            b1c = cp.tile([H, 1], f32)
            nc.sync.dma_start(b1c[:], b1_d[:])
            b2c = cp.tile([H, 1], f32)
            nc.sync.dma_start(b2c[:], b2_d[:])
            ba12 = cp.tile([128, 1], f32)
            nc.sync.dma_start(ba12[:], ba12_d[:])
            scol = cp.tile([128, 1], f32)
            nc.sync.dma_start(scol[:], scol_d[:])
            sbcol = cp.tile([128, 1], f32)
            nc.sync.dma_start(sbcol[:], sbcol_d[:])

            xt = cp.tile([128, 4 * N], bf16)
            for c in range(4):
                for hh in range(2):
                    eng = nc.sync if (2 * c + hh) % 2 == 0 else nc.gpsimd
                    eng.dma_start(
                        xt[:, c * N + hh * 512:c * N + (hh + 1) * 512],
                        xT_d[c * 128:(c + 1) * 128, hh * 512:(hh + 1) * 512])
            xbt = cp.tile([128, 4 * BLK], bf16)
            for c in range(4):
                eng = nc.sync if c % 2 == 0 else nc.gpsimd
                eng.dma_start(xbt[:, c * BLK:(c + 1) * BLK],
                              xbT_d[c * 128:(c + 1) * 128, :])

            # ---- stage 0, j-side: e_jT / e_iT over all N, duplicated 2x on
            # partitions (rows 0-63 and 64-127 both hold the 64 h-dims) ----
            h1T = cp.tile([H, N], bf16)
            hT = cp.tile([H, N], bf16)
            ejT2 = cp.tile([128, N], f32)
            eiT2 = cp.tile([128, N], f32)
            for jh in range(2):
                s = slice(512 * jh, 512 * (jh + 1))
                ps1 = ps0.tile([H, 512], f32, tag="s0")
                for c in range(4):
                    nc.tensor.matmul(ps1[:], w1t[:, c * H:(c + 1) * H],
                                     xt[:, c * N + 512 * jh: c * N + 512 * (jh + 1)],
                                     start=(c == 0), stop=(c == 3))
                nc.scalar.activation(h1T[:, s], ps1[:], AF.Relu, bias=b1c[:])
                ps2 = ps0.tile([H, 512], f32, tag="s0")
                nc.tensor.matmul(ps2[:], w2t[:], h1T[:, s], start=True, stop=True)
                nc.scalar.activation(hT[:, s], ps2[:], AF.Identity, bias=b2c[:])
                psj = ps0.tile([128, 512], f32, tag="s0")
                nc.tensor.matmul(psj[:], wa1hi[:], hT[:, s], start=True, stop=True)
                nc.vector.tensor_copy(ejT2[:, s], psj[:])
                psi = ps0.tile([128, 512], f32, tag="s0")
                nc.tensor.matmul(psi[:], wa1lo[:], hT[:, s], start=True, stop=True)
                nc.vector.tensor_copy(eiT2[:, s], psi[:])

            # ---- stage 0, i-side (this core's 128 rows, from xbT) ----
            h1Tb = cp.tile([H, BLK], bf16)
            hTb = cp.tile([H, BLK], bf16)
            eiTb2p = cp.tile([128, BLK], f32)
            ejTb2p = cp.tile([128, BLK], f32)
            psb1 = ps0.tile([H, BLK], f32, tag="s0")
            for c in range(4):
                nc.tensor.matmul(psb1[:], w1t[:, c * H:(c + 1) * H],
                                 xbt[:, c * BLK:(c + 1) * BLK],
                                 start=(c == 0), stop=(c == 3))
            nc.scalar.activation(h1Tb[:], psb1[:], AF.Relu, bias=b1c[:])
            psb2 = ps0.tile([H, BLK], f32, tag="s0")
            nc.tensor.matmul(psb2[:], w2t[:], h1Tb[:], start=True, stop=True)
            nc.scalar.activation(hTb[:], psb2[:], AF.Identity, bias=b2c[:])
            psbe = ps0.tile([128, BLK], f32, tag="s0")
            nc.tensor.matmul(psbe[:], wa1lo[:], hTb[:], start=True, stop=True)
            nc.scalar.activation(eiTb2p[:], psbe[:], AF.Identity, bias=ba12[:])
            psbe2 = ps0.tile([128, BLK], f32, tag="s0")
            nc.tensor.matmul(psbe2[:], wa1hi[:], hTb[:], start=True, stop=True)
            nc.scalar.activation(ejTb2p[:], psbe2[:], AF.Identity, bias=ba12[:])

            # bias columns: biasA[:, b] = [e_i(2b)+ba1 ; e_i(2b+1)+ba1]
            biasA = cp.tile([128, 64], f32)
            biasB = cp.tile([128, 64], f32)
            nc.vector.tensor_copy(biasA[0:64, :], eiTb2p[0:64, 0:128:2])
            nc.vector.tensor_copy(biasA[64:128, :], eiTb2p[64:128, 1:128:2])
            nc.vector.tensor_copy(biasB[0:64, :], ejTb2p[0:64, 0:128:2])
            nc.vector.tensor_copy(biasB[64:128, :], ejTb2p[64:128, 1:128:2])

            ps0_cm.__exit__(None, None, None)
            psM_cm = tc.tile_pool(name="psumM", bufs=2, space="PSUM")
            psM = psM_cm.__enter__()

            # persistent staging for all 16 quads' sigmoid outputs
            sig_all = cp.tile([128, 16 * N], f32)

            # ---- main loop: 16 quads x 4 col-groups x (A,B) passes ----
            prod_idx = 0

            def producer(out_t, in_t, bias_col):
                nonlocal prod_idx
                kind = PROD_PATTERN[prod_idx % len(PROD_PATTERN)]
                if kind == "a":
                    nc.scalar.activation(out_t[:], in_t[:], AF.Relu,
                                         bias=bias_col)
                else:
                    nc.vector.tensor_scalar(out_t[:], in_t[:], bias_col, 0.0,
                                            OP.add, OP.max)
                prod_idx += 1

            pq = None
            for q in range(16):
                if q % 2 == 0:
                    pq = psM.tile([128, 2 * N], f32, tag="pq")
                base = (q % 2) * N
                pres = []
                for g in range(4):
                    b = 4 * q + g
                    preA = pp.tile([128, N], bf16, tag="pre")
                    producer(preA, ejT2, biasA[:, b:b + 1])
                    preB = pp.tile([128, N], bf16, tag="pre")
                    producer(preB, eiT2, biasB[:, b:b + 1])
                    pres.append((preA, preB))
                # interleave col groups for PE concurrency; A starts the
                # accumulation group of its (col-group, bank), B closes it
                # sim's group tracker is partition-base blind -> false
                # collisions across col groups; HW has_written is
                # per-partition and each 512-f32 MM consumes its whole
                # 2KB zero region, so skipping the check is sound here
                for jh in range(2):
                    s = slice(512 * jh, 512 * (jh + 1))
                    so = slice(base + 512 * jh, base + 512 * (jh + 1))
                    for g in range(4):
                        nc.tensor.matmul(pq[32 * g:32 * g + 32, so], wa2c[:],
                                         pres[g][0][:, s], start=True,
                                         stop=False, tile_position=(0, 32 * g),
                                         skip_group_check=True)
                for jh in range(2):
                    s = slice(512 * jh, 512 * (jh + 1))
                    so = slice(base + 512 * jh, base + 512 * (jh + 1))
                    for g in range(4):
                        nc.tensor.matmul(pq[32 * g:32 * g + 32, so], wa2c[:],
                                         pres[g][1][:, s], start=False,
                                         stop=True, tile_position=(0, 32 * g),
                                         skip_group_check=True)
                if q % 2 == 1:
                    nc.scalar.activation(sig_all[:, (q - 1) * N:(q + 1) * N],
                                         pq[:], AF.Sigmoid,
                                         bias=sbcol[:], scale=scol[:])
                # flush finished output quarters; spread the single-partition
                # reads across idle engine DMA queues
                if q % 4 == 3:
                    qt = q // 4
                    h0 = qt * 4 * N
                    dq = [nc.sync, nc.gpsimd]
                    for k, (g, m) in enumerate(
                            (g, m) for g in range(4) for m in range(2)):
                        srow = sig_all[32 * g + m:32 * g + m + 1,
                                       h0:h0 + 4 * N]
                        dst = adj_d[qt * 32 + 2 * g + m:
                                    qt * 32 + 32:8, :]
                        dq[k % 2].dma_start(dst, srow)
            psM_cm.__exit__(None, None, None)

    nc.compile()
    return nc


def _host_prep(node_features, W1, b1, W2, b2, Wa1, ba1, wa2, ba2, temperature):
    """Host-side input layout prep (transposes / tiling / scalar folding)."""
    import ml_dtypes

    x = np.asarray(node_features, np.float32)
    W1 = np.asarray(W1, np.float32)
    W2 = np.asarray(W2, np.float32)
    Wa1 = np.asarray(Wa1, np.float32)
    b1 = np.asarray(b1, np.float32)
    b2 = np.asarray(b2, np.float32)
    ba1 = np.asarray(ba1, np.float32)
    wa2 = np.asarray(wa2, np.float32)
    ba2 = np.float32(ba2)
    t = float(np.clip(np.float32(temperature), 0.1, 5.0))

    xT = np.ascontiguousarray(x.T)                           # [F, N]
    wa1lo = np.ascontiguousarray(np.tile(Wa1[:H], (1, 2)))   # [64, 128]
    wa1hi = np.ascontiguousarray(np.tile(Wa1[H:], (1, 2)))   # [64, 128]
    wa2c = np.zeros((128, 32), np.float32)
    wa2c[0:64, 0] = wa2
    wa2c[64:128, 1] = wa2
    ba12 = np.tile(ba1, 2).reshape(128, 1)
    scol = np.full((128, 1), 0.5 / t, np.float32)
    sbcol = np.full((128, 1), ba2 / t, np.float32)

    common = {
        "xT": xT.astype(ml_dtypes.bfloat16),
        "w1": W1.astype(ml_dtypes.bfloat16),
        "w2": W2.astype(ml_dtypes.bfloat16),
        "wa1lo": wa1lo.astype(ml_dtypes.bfloat16),
        "wa1hi": wa1hi.astype(ml_dtypes.bfloat16),
        "wa2c": wa2c.astype(ml_dtypes.bfloat16),
        "b1c": b1.reshape(H, 1),
        "b2c": b2.reshape(H, 1),
        "ba12": ba12.astype(np.float32),
        "scol": scol,
        "sbcol": sbcol,
    }
    in_maps = []
    for c in range(NCORES):
        m = dict(common)
        m["xbT"] = np.ascontiguousarray(
            xT[:, c * BLK:(c + 1) * BLK]).astype(ml_dtypes.bfloat16)
        in_maps.append(m)
    return in_maps


def kernel(node_features, W1, b1, W2, b2, Wa1, ba1, wa2, ba2, temperature):
    from concourse.bass_utils import run_bass_kernel_spmd

    if "nc" not in _cache:
        _cache["nc"] = _build_program()
    nc = _cache["nc"]

    in_maps = _host_prep(node_features, W1, b1, W2, b2, Wa1, ba1, wa2, ba2,
                         temperature)
    res = run_bass_kernel_spmd(nc, in_maps, list(range(NCORES)))
    adj = np.concatenate([res.results[c]["adj_blk"] for c in range(NCORES)],
                         axis=0)
    loss = np.float32(SPARSITY_WEIGHT) * np.mean(np.abs(adj), dtype=np.float32)
    return adj, np.float32(loss)
